# revision 7
# baseline (speedup 1.0000x reference)
"""CrossAttention kernel for 8 Trainium2 NeuronCores (Bass/Tile).

Problem (hardcoded): x [4,2048,1024] f32, context [4,2048,1024] f32,
mask [4,2048] bool, Wq/Wk/Wv [1024,512], Wo [512,1024], bo [1024].
8 heads x 64 dim, scale 1/8, out = softmax(q k^T * s + maskbias) v @ Wo + bo.

Sharding: core c -> (batch b = c//2, head-group hg = c%2 of 4 heads).
Each core computes a partial output [2048,1024] (its 4 heads through its
256-row slice of Wo); the host sums core pairs and adds bo.

Device-side layout trick: everything is computed in "transposed" form so
no on-device transposes are needed:
  qT/kT = W^T @ x^T come out of the projection matmul as [d, rows].
  sim is computed as simT [j, i]  (lhsT=kT tile, rhs=qT tile), so the
  context mask/padding bias is per-partition -> fused into the ACT exp
  (exp(sim*scale + bias)) together with the attention scale.
  PV uses expT directly as the moving operand with v' = [v | ones] as the
  stationary one; the ones column yields the softmax denominator for free.
  The PV output [d, i] is exactly the lhsT the Wo projection needs.

The context rows where mask=False are removed on the host (their softmax
weight is exactly zero), and the remainder padded to a multiple of 128
with bias -1e30 rows.

Pipeline structure (per body):
  - attention inner loop is software-pipelined: sim(jt+1) is emitted
    before PV(jt) so the PE never stalls on the ACT exp chain.
  - out-projection uses its own PSUM banks (acc tags) so it doesn't
    steal the sim double-buffers from the next attention call.
  - PSUM budget: sim [128,1024]x2 bufs = 4 banks, acc0/acc1 [128,512]
    x2 bufs = 4 banks -> 8 exactly.
"""

import math

import numpy as np
import ml_dtypes

BF16 = ml_dtypes.bfloat16

B, N, DIM = 4, 2048, 1024
HEADS, DH = 8, 64
INNER = HEADS * DH  # 512
HG = INNER // 2  # 256 per head-group

_PROGRAMS: dict[tuple, object] = {}


def _build_program(m_pad: int, repeats: int = 1):
    import concourse.tile as tile
    from concourse import bacc, mybir

    f32 = mybir.dt.float32
    bf16 = mybir.dt.bfloat16
    Exp = mybir.ActivationFunctionType.Exp
    mpt = m_pad // 128

    nc = bacc.Bacc("TRN2", target_bir_lowering=False, debug=False)
    xT_d = nc.dram_tensor("xT", [DIM, N], bf16, kind="ExternalInput").ap()
    cT_d = nc.dram_tensor("ctxT", [DIM, m_pad], bf16, kind="ExternalInput").ap()
    wq_d = nc.dram_tensor("wq", [DIM, HG], bf16, kind="ExternalInput").ap()
    wk_d = nc.dram_tensor("wk", [DIM, HG], bf16, kind="ExternalInput").ap()
    wv_d = nc.dram_tensor("wv", [DIM, HG], bf16, kind="ExternalInput").ap()
    wo_d = nc.dram_tensor("wo", [HG, DIM], bf16, kind="ExternalInput").ap()
    bias_d = nc.dram_tensor("bias", [128, mpt], f32, kind="ExternalInput").ap()
    out_d = nc.dram_tensor("out", [N, DIM], bf16, kind="ExternalOutput").ap()

    with tile.TileContext(nc) as tc:
        with tc.tile_pool(name="const", bufs=1) as const, tc.tile_pool(
            name="work", bufs=4
        ) as work, tc.tile_pool(name="outp", bufs=3) as outp:
            xT = const.tile([128, 8, N], bf16)
            cT = const.tile([128, 8, m_pad], bf16)
            wq = const.tile([128, 8, HG], bf16)
            wk = const.tile([128, 8, HG], bf16)
            wv = const.tile([128, 8, HG], bf16)
            wo = const.tile([128, 2, DIM], bf16)
            biasv = const.tile([128, mpt], f32)
            qT = const.tile([128, 2, N], bf16)
            kT = const.tile([128, 2, m_pad], bf16)
            vp = const.tile([128, mpt, 4 * (DH + 1)], bf16)
            oT = const.tile([128, 2, N], bf16)

            # DMA order matters for the pipeline head: weights first (tiny,
            # unblock the projection matmuls), then context (v/k-proj), then
            # x (q-proj is needed later than v/k).
            for kt in range(8):
                s = slice(kt * 128, (kt + 1) * 128)
                nc.sync.dma_start(out=wv[:, kt, :], in_=wv_d[s, :])
                nc.sync.dma_start(out=wk[:, kt, :], in_=wk_d[s, :])
                nc.sync.dma_start(out=wq[:, kt, :], in_=wq_d[s, :])
            nc.sync.dma_start(out=biasv[:, :], in_=bias_d[:, :])
            for kt in range(8):
                s = slice(kt * 128, (kt + 1) * 128)
                nc.sync.dma_start(out=cT[:, kt, :], in_=cT_d[s, :])
            for kt in range(8):
                s = slice(kt * 128, (kt + 1) * 128)
                nc.sync.dma_start(out=xT[:, kt, :], in_=xT_d[s, :])
            nc.sync.dma_start(out=wo[:, 0, :], in_=wo_d[0:128, :])
            nc.sync.dma_start(out=wo[:, 1, :], in_=wo_d[128:256, :])
            nc.vector.memset(vp[:, :, :], 1.0)

            def emit_body(psp):
                # ---- v projection (needed first by attention PV) ----------
                # psum on the acc tags; PSUM->SBUF copies on the (otherwise
                # idle during projections) ACT engine.
                for jt in range(mpt):
                    js = slice(jt * 128, (jt + 1) * 128)
                    ps = psp.tile([128, 512], f32, tag="acc" + str(jt % 2))
                    for kt in range(8):
                        nc.tensor.matmul(
                            ps[:, :HG],
                            lhsT=cT[:, kt, js],
                            rhs=wv[:, kt, :],
                            start=(kt == 0),
                            stop=(kt == 7),
                        )
                    for lh in range(4):
                        nc.scalar.copy(
                            out=vp[:, jt, lh * 65 : lh * 65 + 64],
                            in_=ps[:, lh * 64 : (lh + 1) * 64],
                        )

                def kproj(pr):
                    # matmul psum output is limited to one bank (512 f32);
                    # pack two 512-chunks per [128,1024] psum tile so the
                    # DVE copy still runs at 1024 wide.
                    ws = slice(pr * 128, (pr + 1) * 128)
                    j0 = 0
                    while j0 < m_pad:
                        jl = min(1024, m_pad - j0)
                        ps = psp.tile([128, 1024], f32, tag="sim")
                        for half in range(0, jl, 512):
                            hl = min(512, jl - half)
                            for kt in range(8):
                                nc.tensor.matmul(
                                    ps[:, half : half + hl],
                                    lhsT=wk[:, kt, ws],
                                    rhs=cT[:, kt, j0 + half : j0 + half + hl],
                                    start=(kt == 0),
                                    stop=(kt == 7),
                                )
                        nc.vector.tensor_copy(
                            out=kT[:, pr, j0 : j0 + jl], in_=ps[:, :jl]
                        )
                        j0 += jl

                def qproj(pr, icc):
                    cs = icc * 1024
                    ws = slice(pr * 128, (pr + 1) * 128)
                    ps = psp.tile([128, 1024], f32, tag="sim")
                    for half in range(0, 1024, 512):
                        for kt in range(8):
                            nc.tensor.matmul(
                                ps[:, half : half + 512],
                                lhsT=wq[:, kt, ws],
                                rhs=xT[:, kt, cs + half : cs + half + 512],
                                start=(kt == 0),
                                stop=(kt == 7),
                            )
                    nc.vector.tensor_copy(
                        out=qT[:, pr, cs : cs + 1024], in_=ps[:, :]
                    )

                def attn_pair(ic):
                    # ---- attention, BOTH head pairs interleaved -----------
                    # The two K=64 sim matmuls of a pair land in PE
                    # row-groups 0-1 / 2-3 (auto tile_position from lhsT
                    # base partition 0/64) and run concurrently on HW.
                    # Interleaving pr=0/pr=1 gives the PE a full pair-worth
                    # of independent matmuls to chew on while the ACT exp
                    # of the other pair is in flight, so neither engine
                    # ever stalls on the sim->exp->PV chain.
                    i0 = ic * 512
                    qs = slice(i0, i0 + 512)
                    acc = []
                    for _pr in range(2):
                        acc_e = psp.tile([65, 512], f32, tag="acc0", name=f"acc_e{_pr}")
                        acc_o = psp.tile([65, 512], f32, tag="acc1", name=f"acc_o{_pr}")
                        acc.append((acc_e, acc_o))

                    def sim_mm(pr, jt):
                        js = slice(jt * 128, (jt + 1) * 128)
                        sim = psp.tile([128, 1024], f32, tag="sim")
                        nc.tensor.matmul(
                            sim[:, 0:512],
                            lhsT=kT[0:64, pr, js],
                            rhs=qT[0:64, pr, qs],
                            start=True,
                            stop=True,
                        )
                        nc.tensor.matmul(
                            sim[:, 512:1024],
                            lhsT=kT[64:128, pr, js],
                            rhs=qT[64:128, pr, qs],
                            start=True,
                            stop=True,
                        )
                        return sim

                    def exp_act(jt, sim):
                        ex = work.tile([128, 1024], bf16, tag="exp")
                        nc.scalar.activation(
                            out=ex[:, :],
                            in_=sim[:, :],
                            func=Exp,
                            bias=biasv[:, jt : jt + 1],
                            scale=0.125,
                        )
                        return ex

                    def pv_mm(pr, jt, ex):
                        va = slice((2 * pr) * 65, (2 * pr + 1) * 65)
                        vb = slice((2 * pr + 1) * 65, (2 * pr + 2) * 65)
                        nc.tensor.matmul(
                            acc[pr][0][:, :],
                            lhsT=vp[:, jt, va],
                            rhs=ex[:, 0:512],
                            start=(jt == 0),
                            stop=(jt == mpt - 1),
                        )
                        nc.tensor.matmul(
                            acc[pr][1][:, :],
                            lhsT=vp[:, jt, vb],
                            rhs=ex[:, 512:1024],
                            start=(jt == 0),
                            stop=(jt == mpt - 1),
                        )

                    def norm(pr):
                        # normalize: oT = acc[0:64] * (1/acc[64]) bcast
                        for hh in range(2):
                            a = acc[pr][hh]
                            rc = work.tile([1, 512], f32, tag="recip")
                            nc.vector.reciprocal(out=rc[:, :], in_=a[64:65, :])
                            bc = work.tile([64, 512], f32, tag="bcast")
                            nc.gpsimd.partition_broadcast(bc[:, :], rc[:, :])
                            if hh == 0:
                                nc.vector.tensor_mul(
                                    oT[0:64, pr, qs], a[0:64, :], bc[:, :]
                                )
                            else:
                                st = work.tile([64, 512], bf16, tag="stage")
                                nc.vector.tensor_mul(
                                    st[:, :], a[0:64, :], bc[:, :]
                                )
                                nc.sync.dma_start(
                                    out=oT[64:128, pr, qs], in_=st[:, :]
                                )

                    sim_c = [sim_mm(0, 0), sim_mm(1, 0)]
                    ex_c = [exp_act(0, sim_c[0]), exp_act(0, sim_c[1])]
                    for jt in range(mpt):
                        last = jt + 1 >= mpt
                        for pr in range(2):
                            if not last:
                                sim_n = sim_mm(pr, jt + 1)
                            pv_mm(pr, jt, ex_c[pr])
                            if last:
                                norm(pr)
                            else:
                                ex_c[pr] = exp_act(jt + 1, sim_n)

                # ---- drive attention + interleaved output projection -----
                # After both head pairs finish an i-slice, its rows of oT
                # are final: project + DMA them out while the next i-slice's
                # attention runs, hiding the writeback.  The out-proj psum
                # lives on the acc tags ([128,512] halves) so it never
                # competes with the sim double-buffers.  The q projection
                # for i-chunk pair icc is emitted just in time, so later
                # q chunks overlap earlier attention.
                for ic in range(N // 512):
                    if ic == 0:
                        kproj(0)
                        kproj(1)
                    if ic % 2 == 0:
                        qproj(0, ic // 2)
                        qproj(1, ic // 2)
                    attn_pair(ic)
                    for it in range(ic * 4, ic * 4 + 4):
                        ts_ = slice(it * 128, (it + 1) * 128)
                        ob = outp.tile([128, DIM], bf16, tag="ob")
                        for nh2 in range(2):
                            ns = slice(nh2 * 512, (nh2 + 1) * 512)
                            po = psp.tile([128, 512], f32, tag="acc" + str(nh2))
                            for ck2 in range(2):
                                nc.tensor.matmul(
                                    po[:, :],
                                    lhsT=oT[:, ck2, ts_],
                                    rhs=wo[:, ck2, ns],
                                    start=(ck2 == 0),
                                    stop=(ck2 == 1),
                                )
                            nc.vector.tensor_copy(out=ob[:, ns], in_=po[:, :])
                        nc.gpsimd.dma_start(out=out_d[ts_, :], in_=ob[:, :])

            with tc.tile_pool(name="ps", bufs=2, space="PSUM") as psp:
                for _ in range(repeats):
                    emit_body(psp)

    nc.compile()
    return nc


def _get_program(m_pad: int, repeats: int = 1):
    key = (m_pad, repeats)
    if key not in _PROGRAMS:
        _PROGRAMS[key] = _build_program(m_pad, repeats)
    return _PROGRAMS[key]


def make_in_maps(x, context, mask, Wq, Wk, Wv, Wo):
    """Host-side sharding: returns (m_pad, list of 8 per-core input dicts)."""
    x = np.asarray(x, dtype=np.float32)
    context = np.asarray(context, dtype=np.float32)
    mask = np.asarray(mask)
    idxs = []
    for b in range(B):
        idx = np.nonzero(mask[b])[0]
        if idx.size == 0:
            # all masked -> reference softmax degenerates to uniform over all
            idx = np.arange(context.shape[1])
        idxs.append(idx)
    m_pad = max(128, 128 * math.ceil(max(i.size for i in idxs) / 128))

    wq8 = np.asarray(Wq, dtype=np.float32)
    wk8 = np.asarray(Wk, dtype=np.float32)
    wv8 = np.asarray(Wv, dtype=np.float32)
    wo8 = np.asarray(Wo, dtype=np.float32)

    in_maps = []
    for c in range(8):
        b, hg = c // 2, c % 2
        idx = idxs[b]
        mb = idx.size
        xT = np.ascontiguousarray(x[b].T).astype(BF16)
        cTt = np.zeros((DIM, m_pad), dtype=BF16)
        cTt[:, :mb] = np.ascontiguousarray(context[b][idx].T)
        biasv = np.full((m_pad,), -1e30, dtype=np.float32)
        biasv[:mb] = 0.0
        bias_t = np.ascontiguousarray(biasv.reshape(m_pad // 128, 128).T)
        s = slice(hg * HG, (hg + 1) * HG)
        in_maps.append(
            {
                "xT": xT,
                "ctxT": cTt,
                "bias": bias_t,
                "wq": wq8[:, s].astype(BF16),
                "wk": wk8[:, s].astype(BF16),
                "wv": wv8[:, s].astype(BF16),
                "wo": np.ascontiguousarray(wo8[s, :]).astype(BF16),
            }
        )
    return m_pad, in_maps


def kernel(x, context, mask, Wq, Wk, Wv, Wo, bo):
    from concourse.bass_utils import run_bass_kernel_spmd

    m_pad, in_maps = make_in_maps(x, context, mask, Wq, Wk, Wv, Wo)
    nc = _get_program(m_pad)
    res = run_bass_kernel_spmd(nc, in_maps, core_ids=list(range(8))).results
    out = np.empty((B, N, DIM), dtype=np.float32)
    bo32 = np.asarray(bo, dtype=np.float32)
    for b in range(B):
        out[b] = (
            res[2 * b]["out"].astype(np.float32)
            + res[2 * b + 1]["out"].astype(np.float32)
            + bo32
        )
    return out


# revision 11
# speedup vs baseline: 1.0230x; 1.0230x over previous
"""CrossAttention kernel for 8 Trainium2 NeuronCores (Bass/Tile).

Problem (hardcoded): x [4,2048,1024] f32, context [4,2048,1024] f32,
mask [4,2048] bool, Wq/Wk/Wv [1024,512], Wo [512,1024], bo [1024].
8 heads x 64 dim, scale 1/8, out = softmax(q k^T * s + maskbias) v @ Wo + bo.

Sharding: core c -> (batch b = c//2, head-group hg = c%2 of 4 heads).
Each core computes a partial output [2048,1024] (its 4 heads through its
256-row slice of Wo); the host sums core pairs and adds bo.

Device-side layout trick: everything is computed in "transposed" form so
no on-device transposes are needed:
  qT/kT = W^T @ x^T come out of the projection matmul as [d, rows].
  sim is computed as simT [j, i]  (lhsT=kT tile, rhs=qT tile), so the
  context mask/padding bias is per-partition -> fused into the ACT exp
  (exp(sim*scale + bias)) together with the attention scale.
  PV uses expT directly as the moving operand with v' = [v | ones] as the
  stationary one; the ones column yields the softmax denominator for free.
  The PV output [d, i] is exactly the lhsT the Wo projection needs.

The context rows where mask=False are removed on the host (their softmax
weight is exactly zero), and the remainder padded to a multiple of 128
with bias -1e30 rows.

Pipeline structure (per body):
  - attention inner loop is software-pipelined: sim(jt+1) is emitted
    before PV(jt) so the PE never stalls on the ACT exp chain.
  - out-projection uses its own PSUM banks (acc tags) so it doesn't
    steal the sim double-buffers from the next attention call.
  - PSUM budget: sim [128,1024]x2 bufs = 4 banks, acc0/acc1 [128,512]
    x2 bufs = 4 banks -> 8 exactly.
"""

import math

import numpy as np
import ml_dtypes

BF16 = ml_dtypes.bfloat16

B, N, DIM = 4, 2048, 1024
HEADS, DH = 8, 64
INNER = HEADS * DH  # 512
HG = INNER // 2  # 256 per head-group

_PROGRAMS: dict[tuple, object] = {}


def _build_program(m_pad: int, repeats: int = 1):
    import concourse.tile as tile
    from concourse import bacc, mybir

    f32 = mybir.dt.float32
    bf16 = mybir.dt.bfloat16
    Exp = mybir.ActivationFunctionType.Exp
    mpt = m_pad // 128

    nc = bacc.Bacc("TRN2", target_bir_lowering=False, debug=False)
    xT_d = nc.dram_tensor("xT", [DIM, N], bf16, kind="ExternalInput").ap()
    cT_d = nc.dram_tensor("ctxT", [DIM, m_pad], bf16, kind="ExternalInput").ap()
    wq_d = nc.dram_tensor("wq", [DIM, HG], bf16, kind="ExternalInput").ap()
    wk_d = nc.dram_tensor("wk", [DIM, HG], bf16, kind="ExternalInput").ap()
    wv_d = nc.dram_tensor("wv", [DIM, HG], bf16, kind="ExternalInput").ap()
    wo_d = nc.dram_tensor("wo", [HG, DIM], bf16, kind="ExternalInput").ap()
    bias_d = nc.dram_tensor("bias", [128, mpt], f32, kind="ExternalInput").ap()
    out_d = nc.dram_tensor("out", [N, DIM], bf16, kind="ExternalOutput").ap()

    with tile.TileContext(nc) as tc:
        with tc.tile_pool(name="const", bufs=1) as const, tc.tile_pool(
            name="work", bufs=4
        ) as work, tc.tile_pool(name="outp", bufs=3) as outp:
            xT = const.tile([128, 8, N], bf16)
            cT = const.tile([128, 8, m_pad], bf16)
            wq = const.tile([128, 8, HG], bf16)
            wk = const.tile([128, 8, HG], bf16)
            wv = const.tile([128, 8, HG], bf16)
            wo = const.tile([128, 2, DIM], bf16)
            biasv = const.tile([128, mpt], f32)
            qT = const.tile([128, 2, N], bf16)
            kT = const.tile([128, 2, m_pad], bf16)
            vp = const.tile([128, mpt, 4 * (DH + 1)], bf16)
            oT = const.tile([128, 2, N], bf16)
            ones1 = const.tile([1, DH], bf16)

            # DMA order matters for the pipeline head: weights first (tiny,
            # unblock the projection matmuls), then context (v/k-proj), then
            # x (q-proj is needed later than v/k).
            for kt in range(8):
                s = slice(kt * 128, (kt + 1) * 128)
                nc.sync.dma_start(out=wv[:, kt, :], in_=wv_d[s, :])
                nc.sync.dma_start(out=wk[:, kt, :], in_=wk_d[s, :])
                nc.sync.dma_start(out=wq[:, kt, :], in_=wq_d[s, :])
            nc.sync.dma_start(out=biasv[:, :], in_=bias_d[:, :])
            for kt in range(8):
                s = slice(kt * 128, (kt + 1) * 128)
                nc.sync.dma_start(out=cT[:, kt, :], in_=cT_d[s, :])
            for kt in range(8):
                s = slice(kt * 128, (kt + 1) * 128)
                nc.sync.dma_start(out=xT[:, kt, :], in_=xT_d[s, :])
            nc.sync.dma_start(out=wo[:, 0, :], in_=wo_d[0:128, :])
            nc.sync.dma_start(out=wo[:, 1, :], in_=wo_d[128:256, :])
            nc.vector.memset(vp[:, :, :], 1.0)
            nc.vector.memset(ones1[:, :], 1.0)

            def emit_body(psp):
                # ---- v projection (needed first by attention PV) ----------
                # psum on the acc tags; PSUM->SBUF copies on the (otherwise
                # idle during projections) ACT engine.
                for jt in range(mpt):
                    js = slice(jt * 128, (jt + 1) * 128)
                    ps = psp.tile([128, 512], f32, tag="acc" + str(jt % 2))
                    for kt in range(8):
                        nc.tensor.matmul(
                            ps[:, :HG],
                            lhsT=cT[:, kt, js],
                            rhs=wv[:, kt, :],
                            start=(kt == 0),
                            stop=(kt == 7),
                        )
                    for lh in range(4):
                        nc.scalar.copy(
                            out=vp[:, jt, lh * 65 : lh * 65 + 64],
                            in_=ps[:, lh * 64 : (lh + 1) * 64],
                        )

                def kproj(pr):
                    # matmul psum output is limited to one bank (512 f32);
                    # pack two 512-chunks per [128,1024] psum tile so the
                    # DVE copy still runs at 1024 wide.
                    ws = slice(pr * 128, (pr + 1) * 128)
                    j0 = 0
                    while j0 < m_pad:
                        jl = min(1024, m_pad - j0)
                        ps = psp.tile([128, 1024], f32, tag="sim")
                        for half in range(0, jl, 512):
                            hl = min(512, jl - half)
                            for kt in range(8):
                                nc.tensor.matmul(
                                    ps[:, half : half + hl],
                                    lhsT=wk[:, kt, ws],
                                    rhs=cT[:, kt, j0 + half : j0 + half + hl],
                                    start=(kt == 0),
                                    stop=(kt == 7),
                                )
                        nc.vector.tensor_copy(
                            out=kT[:, pr, j0 : j0 + jl], in_=ps[:, :jl]
                        )
                        j0 += jl

                def qproj(pr, icc):
                    cs = icc * 1024
                    ws = slice(pr * 128, (pr + 1) * 128)
                    ps = psp.tile([128, 1024], f32, tag="sim")
                    for half in range(0, 1024, 512):
                        for kt in range(8):
                            nc.tensor.matmul(
                                ps[:, half : half + 512],
                                lhsT=wq[:, kt, ws],
                                rhs=xT[:, kt, cs + half : cs + half + 512],
                                start=(kt == 0),
                                stop=(kt == 7),
                            )
                    nc.vector.tensor_copy(
                        out=qT[:, pr, cs : cs + 1024], in_=ps[:, :]
                    )

                def attn_pair(ic):
                    # ---- attention, BOTH head pairs interleaved -----------
                    # The two K=64 sim matmuls of a pair land in PE
                    # row-groups 0-1 / 2-3 (auto tile_position from lhsT
                    # base partition 0/64) and run concurrently on HW.
                    # Interleaving pr=0/pr=1 gives the PE a full pair-worth
                    # of independent matmuls to chew on while the ACT exp
                    # of the other pair is in flight, so neither engine
                    # ever stalls on the sim->exp->PV chain.
                    i0 = ic * 512
                    qs = slice(i0, i0 + 512)
                    acc = []
                    for _pr in range(2):
                        acc_e = psp.tile([65, 512], f32, tag="acc0", name=f"acc_e{_pr}")
                        acc_o = psp.tile([65, 512], f32, tag="acc1", name=f"acc_o{_pr}")
                        acc.append((acc_e, acc_o))

                    def sim_mm(pr, jt):
                        js = slice(jt * 128, (jt + 1) * 128)
                        sim = psp.tile([128, 1024], f32, tag="sim")
                        nc.tensor.matmul(
                            sim[:, 0:512],
                            lhsT=kT[0:64, pr, js],
                            rhs=qT[0:64, pr, qs],
                            start=True,
                            stop=True,
                        )
                        nc.tensor.matmul(
                            sim[:, 512:1024],
                            lhsT=kT[64:128, pr, js],
                            rhs=qT[64:128, pr, qs],
                            start=True,
                            stop=True,
                        )
                        return sim

                    def exp_act(jt, sim):
                        ex = work.tile([128, 1024], bf16, tag="exp")
                        nc.scalar.activation(
                            out=ex[:, :],
                            in_=sim[:, :],
                            func=Exp,
                            bias=biasv[:, jt : jt + 1],
                            scale=0.125,
                        )
                        return ex

                    def pv_mm(pr, jt, ex):
                        va = slice((2 * pr) * 65, (2 * pr + 1) * 65)
                        vb = slice((2 * pr + 1) * 65, (2 * pr + 2) * 65)
                        nc.tensor.matmul(
                            acc[pr][0][:, :],
                            lhsT=vp[:, jt, va],
                            rhs=ex[:, 0:512],
                            start=(jt == 0),
                            stop=(jt == mpt - 1),
                        )
                        nc.tensor.matmul(
                            acc[pr][1][:, :],
                            lhsT=vp[:, jt, vb],
                            rhs=ex[:, 512:1024],
                            start=(jt == 0),
                            stop=(jt == mpt - 1),
                        )

                    def norm(pr):
                        # normalize: oT = acc[0:64] * (1/acc[64]) broadcast.
                        # The partition-broadcast of the reciprocal row runs
                        # as a K=1 matmul (ones[1,64]^T @ rc[1,1024]) on the
                        # PE -- the gpsimd partition_broadcast costs ~3.5us
                        # of Q7 overhead per call and was the #1 serial
                        # bottleneck of the whole kernel.
                        rc2 = work.tile([1, 1024], bf16, tag="recip")
                        with nc.allow_low_precision("softmax denom in bf16"):
                            nc.vector.reciprocal(
                                out=rc2[:, 0:512], in_=acc[pr][0][64:65, :]
                            )
                            nc.vector.reciprocal(
                                out=rc2[:, 512:1024], in_=acc[pr][1][64:65, :]
                            )
                        bc = psp.tile([64, 1024], f32, tag="sim", name="bc")
                        nc.tensor.matmul(
                            bc[:, 0:512], lhsT=ones1[:, :], rhs=rc2[:, 0:512],
                            start=True, stop=True,
                        )
                        nc.tensor.matmul(
                            bc[:, 512:1024], lhsT=ones1[:, :],
                            rhs=rc2[:, 512:1024], start=True, stop=True,
                        )
                        # DVE can read only one PSUM operand (acc), so hop
                        # the broadcast rows to SBUF on the ACT engine.
                        bcs = work.tile([64, 1024], f32, tag="bcast")
                        nc.scalar.copy(out=bcs[:, :], in_=bc[:, :])
                        nc.vector.tensor_mul(
                            oT[0:64, pr, qs], acc[pr][0][0:64, :], bcs[:, 0:512]
                        )
                        st = work.tile([64, 512], bf16, tag="stage")
                        nc.vector.tensor_mul(
                            st[:, :], acc[pr][1][0:64, :], bcs[:, 512:1024]
                        )
                        nc.sync.dma_start(
                            out=oT[64:128, pr, qs], in_=st[:, :]
                        )

                    sim_c = [sim_mm(0, 0), sim_mm(1, 0)]
                    ex_c = [exp_act(0, sim_c[0]), exp_act(0, sim_c[1])]
                    for jt in range(mpt):
                        last = jt + 1 >= mpt
                        for pr in range(2):
                            if not last:
                                sim_n = sim_mm(pr, jt + 1)
                            pv_mm(pr, jt, ex_c[pr])
                            if last:
                                norm(pr)
                            else:
                                ex_c[pr] = exp_act(jt + 1, sim_n)

                # ---- drive attention + interleaved output projection -----
                # After both head pairs finish an i-slice, its rows of oT
                # are final: project + DMA them out while the next i-slice's
                # attention runs, hiding the writeback.  The out-proj psum
                # lives on the acc tags ([128,512] halves) so it never
                # competes with the sim double-buffers.  The q projection
                # for i-chunk pair icc is emitted just in time, so later
                # q chunks overlap earlier attention.
                for ic in range(N // 512):
                    if ic == 0:
                        kproj(0)
                        kproj(1)
                    if ic % 2 == 0:
                        qproj(0, ic // 2)
                        qproj(1, ic // 2)
                    attn_pair(ic)
                    for it in range(ic * 4, ic * 4 + 4):
                        ts_ = slice(it * 128, (it + 1) * 128)
                        ob = outp.tile([128, DIM], bf16, tag="ob")
                        for nh2 in range(2):
                            ns = slice(nh2 * 512, (nh2 + 1) * 512)
                            po = psp.tile([128, 512], f32, tag="acc" + str(nh2))
                            for ck2 in range(2):
                                nc.tensor.matmul(
                                    po[:, :],
                                    lhsT=oT[:, ck2, ts_],
                                    rhs=wo[:, ck2, ns],
                                    start=(ck2 == 0),
                                    stop=(ck2 == 1),
                                )
                            nc.vector.tensor_copy(out=ob[:, ns], in_=po[:, :])
                        nc.gpsimd.dma_start(out=out_d[ts_, :], in_=ob[:, :])

            with tc.tile_pool(name="ps", bufs=2, space="PSUM") as psp:
                for _ in range(repeats):
                    emit_body(psp)

    nc.compile()
    return nc


def _get_program(m_pad: int, repeats: int = 1):
    key = (m_pad, repeats)
    if key not in _PROGRAMS:
        _PROGRAMS[key] = _build_program(m_pad, repeats)
    return _PROGRAMS[key]


def make_in_maps(x, context, mask, Wq, Wk, Wv, Wo):
    """Host-side sharding: returns (m_pad, list of 8 per-core input dicts)."""
    x = np.asarray(x, dtype=np.float32)
    context = np.asarray(context, dtype=np.float32)
    mask = np.asarray(mask)
    idxs = []
    for b in range(B):
        idx = np.nonzero(mask[b])[0]
        if idx.size == 0:
            # all masked -> reference softmax degenerates to uniform over all
            idx = np.arange(context.shape[1])
        idxs.append(idx)
    m_pad = max(128, 128 * math.ceil(max(i.size for i in idxs) / 128))

    wq8 = np.asarray(Wq, dtype=np.float32)
    wk8 = np.asarray(Wk, dtype=np.float32)
    wv8 = np.asarray(Wv, dtype=np.float32)
    wo8 = np.asarray(Wo, dtype=np.float32)

    in_maps = []
    for c in range(8):
        b, hg = c // 2, c % 2
        idx = idxs[b]
        mb = idx.size
        xT = np.ascontiguousarray(x[b].T).astype(BF16)
        cTt = np.zeros((DIM, m_pad), dtype=BF16)
        cTt[:, :mb] = np.ascontiguousarray(context[b][idx].T)
        biasv = np.full((m_pad,), -1e30, dtype=np.float32)
        biasv[:mb] = 0.0
        bias_t = np.ascontiguousarray(biasv.reshape(m_pad // 128, 128).T)
        s = slice(hg * HG, (hg + 1) * HG)
        in_maps.append(
            {
                "xT": xT,
                "ctxT": cTt,
                "bias": bias_t,
                "wq": wq8[:, s].astype(BF16),
                "wk": wk8[:, s].astype(BF16),
                "wv": wv8[:, s].astype(BF16),
                "wo": np.ascontiguousarray(wo8[s, :]).astype(BF16),
            }
        )
    return m_pad, in_maps


def kernel(x, context, mask, Wq, Wk, Wv, Wo, bo):
    from concourse.bass_utils import run_bass_kernel_spmd

    m_pad, in_maps = make_in_maps(x, context, mask, Wq, Wk, Wv, Wo)
    nc = _get_program(m_pad)
    res = run_bass_kernel_spmd(nc, in_maps, core_ids=list(range(8))).results
    out = np.empty((B, N, DIM), dtype=np.float32)
    bo32 = np.asarray(bo, dtype=np.float32)
    for b in range(B):
        out[b] = (
            res[2 * b]["out"].astype(np.float32)
            + res[2 * b + 1]["out"].astype(np.float32)
            + bo32
        )
    return out
